# revision 35
# baseline (speedup 1.0000x reference)
"""Trainium2 Bass kernel for nn_AdjacencyMaskedNet.

Reference math (N=4096, I=512, O=512, O_=8 groups, H=2048, GROUP=64):
    for each group g: h_g = relu((x * A_mask[:, g]) @ W1 + b1)
                      y_g = h_g @ W2 + b2
    out[n, j] = y_{col_idx[j]}[n, j]

With the canonical inputs, A_mask[:, g] is the prefix mask over the first
64*(g+1) inputs and col_idx = arange(512) // 64.  Structural wins:

  1. mm1 is computed *incrementally* in PSUM: a_g = a_{g-1} + (next block
     of x) @ (matching W1 rows).  One full-matmul worth of PE column-cycles
     instead of 8.
  2. mm2 only needs the 64 output columns of each group's y: an 8x cut.
  3. The 8 running-sum snapshots per H-block come from FOUR independent
     single-bank chains per hb-pair (hb in {2P, 2P+1} x parity in {even,
     odd}), each taking 4 increments of K=128 (even chains use a
     64-row-shifted DMA view of the same W1/x DRAM; their step 0 is the
     first K=64 half-block):
        odd  chain of hb: B0B1 | B2B3 | B4B5 | B6B7   -> snapshots g=1,3,5,7
        even chain of hb: B0 | B1B2 | B3B4 | B5B6     -> snapshots g=0,2,4,6
  4. mm2 (M=64) instructions alternate PE column halves (even groups ->
     cols 0-63, odd -> 64-127), so consecutive mm2s run concurrently in
     the array (measured ~108ns/matmul at 512 free vs 222 standalone),
     packing two groups per PSUM bank.
  5. PSUM->SBUF relu (the snapshot evacuation) is the co-bottleneck with
     the PE: 65536 columns at ~1.35ns/col.  It is split into narrow
     [128,512] ops balanced 2/2 across ACT and DVE every step so each
     chain's write-after-read stall is one narrow relu (~690ns), hidden
     inside the ~1.35us step window.  (GPSIMD cannot access PSUM.)
  6. Everything is bf16 (x, W1, W2, h): halves DMA and SBUF traffic, no
     PE dtype switches.  PSUM accumulates fp32.  Measured end-to-end
     L2 rel err ~3e-3 (vs 2.4e-3 for f32r mm1).

Sharding: data-parallel over batch, 512 rows per core, 8 cores, no
collectives.  Each core computes outT (O, 512) for its batch shard.
"""

import sys

if "/opt/trn_rl_repo" not in sys.path:
    sys.path.insert(0, "/opt/trn_rl_repo")

import numpy as np
import ml_dtypes

N, I, O, O_, H = 4096, 512, 512, 8, 2048
GROUP = O // O_  # 64
NCORES = 8
NC = N // NCORES  # 512 batch rows per core
HB = H // 128  # 16 H-blocks
NPAIR = O // 128  # 4 psum banks for mm2 (two 64-col groups per bank)
NSTEP = 4

_CACHE = {}


def _canonical_mask():
    g = np.arange(O_)
    return (np.arange(I)[:, None] < (g[None, :] + 1) * (I // O_)).astype(np.float32)


def _build_program(with_bias, with_bias2):
    """Build + compile the Bass program once per process."""
    import concourse.tile as tile
    from concourse import bacc, mybir

    f32 = mybir.dt.float32
    bf16 = mybir.dt.bfloat16
    Relu = mybir.ActivationFunctionType.Relu
    Ident = mybir.ActivationFunctionType.Identity
    Alu = mybir.AluOpType

    nc = bacc.Bacc("TRN2", target_bir_lowering=False, debug=False, num_devices=NCORES)

    xt = nc.dram_tensor("xt", [I, NC], bf16, kind="ExternalInput").ap()
    w1 = nc.dram_tensor("w1", [I, H], bf16, kind="ExternalInput").ap()
    w2 = nc.dram_tensor("w2", [H, O], bf16, kind="ExternalInput").ap()
    b1t = nc.dram_tensor("b1t", [128, HB], f32, kind="ExternalInput").ap()
    b2t = nc.dram_tensor("b2t", [128, NPAIR], f32, kind="ExternalInput").ap()
    ot = nc.dram_tensor("ot", [O, NC], f32, kind="ExternalOutput").ap()

    with tile.TileContext(nc) as tc:
        with (
            tc.tile_pool(name="const", bufs=1) as cp,
            tc.tile_pool(name="hpool", bufs=24) as hp,
            tc.tile_pool(name="opool", bufs=1) as op,
            tc.tile_pool(name="ps1", bufs=4, space="PSUM") as ps1,
            tc.tile_pool(name="ps2", bufs=1, space="PSUM") as ps2,
        ):
            # ---- tiles for resident inputs.  Dependencies are per-TILE, so
            # early-needed data is split into separate tiles at the
            # granularity consumers actually touch: the q=0 W1 tiles split
            # into pair-0 (cols 0:256) / pair-1 (256:512) halves, and the
            # w2 tiles for hb<4 into group 0-3 / 4-7 halves.
            xnat = [cp.tile([128, NC], bf16, name=f"xn{m}") for m in range(4)]
            xsh = [cp.tile([128, NC], bf16, name=f"xh{m}") for m in range(3)]
            w1nat = [
                [
                    (
                        [cp.tile([128, 256], bf16, name=f"w1n{m}_0a"),
                         cp.tile([128, 256], bf16, name=f"w1n{m}_0b")]
                        if q == 0
                        else cp.tile([128, 512], bf16, name=f"w1n{m}_{q}")
                    )
                    for q in range(4)
                ]
                for m in range(4)
            ]
            w1sh = [
                [
                    (
                        [cp.tile([128, 256], bf16, name=f"w1s{m}_0a"),
                         cp.tile([128, 256], bf16, name=f"w1s{m}_0b")]
                        if q == 0
                        else cp.tile([128, 512], bf16, name=f"w1s{m}_{q}")
                    )
                    for q in range(4)
                ]
                for m in range(3)
            ]
            w2sb = [
                (
                    [cp.tile([128, 256], bf16, name=f"w2_{k}a"),
                     cp.tile([128, 256], bf16, name=f"w2_{k}b")]
                    if k < 4
                    else cp.tile([128, O], bf16, name=f"w2_{k}")
                )
                for k in range(HB)
            ]
            b1sb = cp.tile([128, HB], f32, name="b1sb")
            b2sb = cp.tile([128, NPAIR], f32, name="b2sb")

            # zero operands for the has_written-initializing dummy matmuls.
            # Emitted on Vector BEFORE any DMA issue so the mm2 bank inits
            # are unblocked at t~0.
            z1 = cp.tile([1, 128], bf16, name="z1")
            nc.vector.memset(z1[:], 0.0)
            z2 = cp.tile([1, NC], bf16, name="z2")
            nc.vector.memset(z2[:], 0.0)


            # ---- DMAs.  Descriptor issue is ~0.6us of engine time per
            # dma_start, so the early tiles (everything pair 0-1 touches:
            # all of x, the q=0 W1 tiles, w2 blocks 0-3 -- ~1.8MB needed
            # within the first few windows) are fanned out across five
            # engine queues; the q>=1 stream alternates Sync/GpSimd.
            def w1n_src(m, q):
                return w1[m * 128 : (m + 1) * 128, q * 512 : (q + 1) * 512]

            def w1s_src(m, q):
                return w1[64 + m * 128 : 64 + (m + 1) * 128, q * 512 : (q + 1) * 512]

            def w1n0_src(m, half):
                return w1[m * 128 : (m + 1) * 128, half * 256 : (half + 1) * 256]

            def w1s0_src(m, half):
                return w1[64 + m * 128 : 64 + (m + 1) * 128, half * 256 : (half + 1) * 256]

            def xn_src(m):
                return xt[m * 128 : (m + 1) * 128, :]

            def xs_src(m):
                return xt[64 + m * 128 : 64 + (m + 1) * 128, :]

            def w2_src(k):
                return w2[k * 128 : (k + 1) * 128, :]

            def w2h_src(k, half):
                return w2[k * 128 : (k + 1) * 128, half * 256 : (half + 1) * 256]

            # Early tiles in strict window-need order (window (P,s) uses
            # xnat[s] + the pair-P half of w1n[s][0] for the odd chain,
            # xsh[s-1]/w1s[s-1][0] for the even chain; mm2(P,s) at window+1
            # uses the s<2 / s>=2 half of w2sb[2P..2P+1]), round-robined
            # across the three DMA-capable engines.
            early = [
                (xnat[0][:], xn_src(0)), (w1nat[0][0][0][:], w1n0_src(0, 0)),
                (xnat[1][:], xn_src(1)), (w1nat[1][0][0][:], w1n0_src(1, 0)),
                (xsh[0][:], xs_src(0)), (w1sh[0][0][0][:], w1s0_src(0, 0)),
                (w2sb[0][0][:], w2h_src(0, 0)), (w2sb[1][0][:], w2h_src(1, 0)),
                (xnat[2][:], xn_src(2)), (w1nat[2][0][0][:], w1n0_src(2, 0)),
                (xsh[1][:], xs_src(1)), (w1sh[1][0][0][:], w1s0_src(1, 0)),
                (xnat[3][:], xn_src(3)), (w1nat[3][0][0][:], w1n0_src(3, 0)),
                (xsh[2][:], xs_src(2)), (w1sh[2][0][0][:], w1s0_src(2, 0)),
                (w2sb[0][1][:], w2h_src(0, 1)), (w2sb[1][1][:], w2h_src(1, 1)),
                (w1nat[0][0][1][:], w1n0_src(0, 1)), (w1nat[1][0][1][:], w1n0_src(1, 1)),
                (w1sh[0][0][1][:], w1s0_src(0, 1)),
                (w2sb[2][0][:], w2h_src(2, 0)), (w2sb[3][0][:], w2h_src(3, 0)),
                (w1nat[2][0][1][:], w1n0_src(2, 1)), (w1sh[1][0][1][:], w1s0_src(1, 1)),
                (w1nat[3][0][1][:], w1n0_src(3, 1)), (w1sh[2][0][1][:], w1s0_src(2, 1)),
                (w2sb[2][1][:], w2h_src(2, 1)), (w2sb[3][1][:], w2h_src(3, 1)),
            ]
            if with_bias:
                early.append((b1sb[:], b1t[:]))
            if with_bias2:
                early.append((b2sb[:], b2t[:]))
            # DGE ring backpressure makes dma_start block the issuing engine
            # until earlier transfers drain, so Scalar (which must run relus
            # from ~10us) only issues the two small first-window W1 halves;
            # everything else alternates Sync/GpSimd, which have slack.
            _flip = [0]
            for i, (dst, src) in enumerate(early):
                if i in (1, 5):
                    nc.scalar.dma_start(dst, src)
                else:
                    eng = nc.sync if _flip[0] % 2 == 0 else nc.gpsimd
                    _flip[0] += 1
                    eng.dma_start(dst, src)

            _di = [0]

            def dma(dst, src):
                eng = nc.sync if _di[0] % 2 == 0 else nc.gpsimd
                _di[0] += 1
                eng.dma_start(dst, src)

            for q in range(1, 4):
                for m in range(4):
                    dma(w1nat[m][q][:], w1n_src(m, q))
                for m in range(3):
                    dma(w1sh[m][q][:], w1s_src(m, q))
                for k in range(q * 4, (q + 1) * 4):
                    dma(w2sb[k][:], w2_src(k))

            def w1_slice(per_m, hb, rows=slice(0, 128)):
                q, hq = hb // 4, hb % 4
                if q == 0:
                    t = per_m[0][hq // 2]
                    return t[rows, (hq % 2) * 128 : (hq % 2) * 128 + 128]
                return per_m[q][rows, hq * 128 : (hq + 1) * 128]

            def w2_slice(hb, g):
                if hb < 4:
                    return w2sb[hb][g // 4][:, (g % 4) * GROUP : (g % 4 + 1) * GROUP]
                return w2sb[hb][:, g * GROUP : (g + 1) * GROUP]

            # ---- mm2 accumulators: 4 banks, two 64-col groups per bank.
            # start=True on any matmul clears has_written for the WHOLE bank
            # (nuking the other group), so init each bank once with a zero
            # matmul and accumulate with start=False afterwards.  These run
            # during the initial DMA wait.
            mm2ps = [ps2.tile([128, NC], f32, name=f"mm2_{t}") for t in range(NPAIR)]
            for t in range(NPAIR):
                nc.tensor.matmul(
                    mm2ps[t][:], z1[:], z2[:], start=True, stop=False,
                    skip_group_check=True,
                )


            # mm1 increment operands for (parity, step, hb):
            #   odd  chain step s: lhsT = w1nat[s][q] slice, rhs = xnat[s]
            #   even chain step 0: lhsT = w1nat[0][q][0:64] slice, rhs = xnat[0][0:64]
            #   even chain step s>0: lhsT = w1sh[s-1][q] slice, rhs = xsh[s-1]
            def mm1_ops(parity, s, hb):
                if parity == 1:
                    return w1_slice(w1nat[s], hb), xnat[s][:]
                if s == 0:
                    return w1_slice(w1nat[0], hb, slice(0, 64)), xnat[0][0:64, :]
                return w1_slice(w1sh[s - 1], hb), xsh[s - 1][:]

            def emit_relu(h, acc, hb, use_act):
                if with_bias:
                    if use_act:
                        nc.scalar.activation(
                            h[:], acc[:], Relu, bias=b1sb[:, hb : hb + 1]
                        )
                    else:
                        nc.vector.tensor_scalar(
                            h[:], acc[:], b1sb[:, hb : hb + 1], 0.0, Alu.add, Alu.max
                        )
                else:
                    if use_act:
                        nc.scalar.activation(h[:], acc[:], Relu)
                    else:
                        nc.vector.tensor_scalar_max(h[:], acc[:], 0.0)

            def evac(s):
                # split each bank's PSUM->SBUF evacuation into halves on
                # ACT and DVE concurrently, with parallel half-DMAs out.
                osA = op.tile([128, 256], f32, name=f"os{s}a")
                osB = op.tile([128, 256], f32, name=f"os{s}b")
                if with_bias2:
                    nc.scalar.activation(
                        osA[:], mm2ps[s][:, 0:256], Ident, bias=b2sb[:, s : s + 1]
                    )
                    nc.vector.tensor_scalar_add(
                        osB[:], mm2ps[s][:, 256:512], b2sb[:, s : s + 1]
                    )
                else:
                    nc.scalar.copy(osA[:], mm2ps[s][:, 0:256])
                    nc.vector.tensor_copy(osB[:], mm2ps[s][:, 256:512])
                nc.sync.dma_start(ot[s * 128 : (s + 1) * 128, 0:256], osA[:])
                nc.sync.dma_start(ot[s * 128 : (s + 1) * 128, 256:512], osB[:])

            # mm2 for the window (pair, s): group g = 2s+p of hb = 2*pair+w,
            # into psum bank s, column half p.  Emitting p=0/p=1 back-to-back
            # makes them run concurrently in opposite PE column halves.
            def emit_mm2(pair, s, hs):
                for w in range(2):
                    hb = 2 * pair + w
                    for p in range(2):
                        g = 2 * s + p
                        nc.tensor.matmul(
                            mm2ps[s][64 * p : 64 * p + 64, :],
                            w2_slice(hb, g),
                            hs[w][p][:],
                            start=False,
                            stop=(hb == HB - 1 and p == 1),
                            skip_group_check=True,
                        )
                if pair == HB // 2 - 1:
                    evac(s)

            # ---- main loop: windows (pair, s).  Per window: 4 mm1s (one per
            # chain), 4 narrow relus (2 ACT + 2 DVE), then the 4 mm2s of the
            # previous window (lag 1 keeps PE fed while relus drain).
            prev = None
            for pair in range(HB // 2):
                accs = [
                    [ps1.tile([128, NC], f32, tag="acc", name="acc") for _ in range(2)]
                    for _ in range(2)
                ]
                for s in range(NSTEP):
                    hcur = [[None] * 2 for _ in range(2)]
                    for w in range(2):
                        hb = 2 * pair + w
                        for p in range(2):
                            lhsT, rhs = mm1_ops(p, s, hb)
                            nc.tensor.matmul(
                                accs[w][p][:], lhsT, rhs,
                                start=(s == 0), stop=(s == NSTEP - 1),
                            )
                    for w in range(2):
                        hb = 2 * pair + w
                        for p in range(2):
                            h = hp.tile([128, NC], bf16, tag="h", name="h")
                            emit_relu(h, accs[w][p], hb, use_act=((w + p) % 2 == 0))
                            hcur[w][p] = h
                    if prev is not None:
                        emit_mm2(*prev)
                    prev = (pair, s, hcur)
            emit_mm2(*prev)

    nc.compile()
    return nc


def _get_program(with_bias, with_bias2):
    key = ("nc", with_bias, with_bias2)
    if key not in _CACHE:
        _CACHE[key] = _build_program(with_bias, with_bias2)
    return _CACHE[key]


def _run_on_hw(x, W1, b1, W2p, b2p, trace=False, trace_cores=None):
    """Run the bass kernel on 8 cores.  W2p/b2p already column-permuted so
    group g owns contiguous output columns [64g, 64g+64)."""
    from concourse.bass_utils import run_bass_kernel_spmd

    with_bias = bool(np.any(b1 != 0.0))
    with_bias2 = bool(np.any(b2p != 0.0))
    nc = _get_program(with_bias, with_bias2)

    bf = ml_dtypes.bfloat16
    w2bf = np.ascontiguousarray(W2p.astype(bf))
    w1bf = np.ascontiguousarray(W1.astype(bf))
    b1t = np.ascontiguousarray(b1.reshape(HB, 128).T.astype(np.float32))
    b2t = np.ascontiguousarray(b2p.reshape(NPAIR, 128).T.astype(np.float32))

    in_maps = []
    for c in range(NCORES):
        xtc = np.ascontiguousarray(x[c * NC : (c + 1) * NC, :].T.astype(bf))  # (I, NC)
        in_maps.append({"xt": xtc, "w1": w1bf, "w2": w2bf, "b1t": b1t, "b2t": b2t})

    kwargs = {}
    if trace:
        kwargs["trace"] = True
        if trace_cores is not None:
            kwargs["trace_cores"] = trace_cores
    res = run_bass_kernel_spmd(nc, in_maps, core_ids=list(range(NCORES)), **kwargs)

    outT = np.stack([res.results[c]["ot"] for c in range(NCORES)])  # (8, O, NC)
    out = np.ascontiguousarray(np.transpose(outT, (0, 2, 1))).reshape(N, O)
    return out, res


def _reference_numpy(x, W1, b1, W2, b2, A_mask, col_idx):
    """Exact fallback for non-canonical adjacency inputs."""
    n = x.shape[0]
    o_ = A_mask.shape[1]
    out = np.empty((n, W2.shape[1]), dtype=np.float32)
    cols_done = np.zeros(W2.shape[1], dtype=bool)
    for g in range(o_):
        cols = np.nonzero(col_idx == g)[0]
        if len(cols) == 0:
            continue
        h = np.maximum(0.0, (x * A_mask[:, g][None, :]) @ W1 + b1)
        out[:, cols] = h @ W2[:, cols] + b2[cols]
        cols_done[cols] = True
    out[:, ~cols_done] = 0.0
    return out


def kernel(x, W1, b1, W2, b2, A_mask, col_idx, _trace=False, _trace_cores=None):
    x = np.asarray(x, dtype=np.float32)
    W1 = np.asarray(W1, dtype=np.float32)
    b1 = np.asarray(b1, dtype=np.float32)
    W2 = np.asarray(W2, dtype=np.float32)
    b2 = np.asarray(b2, dtype=np.float32)
    A_mask = np.asarray(A_mask, dtype=np.float32)
    col_idx_np = np.asarray(col_idx).astype(np.int64)

    canonical = (
        x.shape == (N, I)
        and W1.shape == (I, H)
        and W2.shape == (H, O)
        and A_mask.shape == (I, O_)
        and col_idx_np.shape == (O,)
        and np.array_equal(A_mask, _canonical_mask())
        and np.all(np.bincount(col_idx_np, minlength=O_) == GROUP)
        and np.all(col_idx_np >= 0)
        and np.all(col_idx_np < O_)
    )
    if not canonical:
        return _reference_numpy(x, W1, b1, W2, b2, A_mask, col_idx_np)

    perm = np.argsort(col_idx_np, kind="stable")  # cols for group 0, then 1, ...
    W2p = W2[:, perm]
    b2p = b2[perm]
    out_p, res = _run_on_hw(x, W1, b1, W2p, b2p, trace=_trace, trace_cores=_trace_cores)
    out = np.empty_like(out_p)
    out[:, perm] = out_p
    if _trace:
        return out, res
    return out
